# revision 30
# baseline (speedup 1.0000x reference)
"""DGI (2-layer GCN encoder + bilinear discriminator) on 8 TRN2 NeuronCores.

Sharding: nodes (and edges by destination) across 8 cores. Per layer, each
core computes its shard of h @ W (pos|neg fused on the feature axis), the
full feature table is AllGathered in 4 bucket slices (each <=32K rows so
int16 gather indices work), and the segment-sum aggregation for the core's
destination shard runs as dma_gather row gathers + one-DVE-op selection
matrices + PSUM-accumulated matmuls.

Key perf structure:
- Gathers are GROUPED over GROUPG destination blocks per bucket, amortizing
  the ~2-3us per-gather SWDGE fixed cost. Interior padding inside a grouped
  window fetches row 0 (multiplied by enorm=0); only the window tail is
  truncated via a per-core count register.
- Both layers' tables are bf16 (512B/1KB rows), selection matrices are bf16
  built from a bf16 iota for 2x DVE rate.
- The per-bucket AllGathers are kicked as soon as the producing blocks are
  written, so collectives overlap phase-A compute (layer 1) and the layer-1
  aggregation (layer 2).
"""
import sys
sys.path.insert(0, "/opt/trn_rl_repo")

import numpy as np
import ml_dtypes
import concourse.bass as bass
import concourse.bacc as bacc
import concourse.tile as tile
from concourse import bass_utils, mybir
from concourse.masks import make_identity
from concourse.tile import add_dep_helper

F32 = mybir.dt.float32
BF16 = mybir.dt.bfloat16
F8 = mybir.dt.float8e3
I16 = mybir.dt.int16
I32 = mybir.dt.int32

GATHER_BF16 = True   # feature tables gathered by edges + collectives in bf16
BUCK = 4             # table buckets (each <= 32K rows; aligned to NP/4)
GROUPG = 3           # destination blocks per grouped gather window
MGG = 4              # gather groups per meta-load window
GAT_BUFS = 14
WARM_WIN = 8         # first windows fetch full (0-padded) so slots start finite
TRUNC_GATHER = True  # -1-padded window tails truncated by per-core count regs
LOCAL_SIM = False    # replace collectives with local copies (TimelineSim)
PSA_BUFS = 3
MP_BUFS = 8


def _cdiv(a, b):
    return -(-a // b)


# ----------------------------------------------------------------------------
# host-side preprocessing
# ----------------------------------------------------------------------------

def _prep(x, edge_index, perm, C):
    N, IN = x.shape
    E = edge_index.shape[1]
    assert N % C == 0
    SH = N // C
    NB = _cdiv(SH, 128)
    NP = NB * 128
    NPAD = NP * C
    assert NP % BUCK == 0
    QB = NP // BUCK          # local rows per bucket slice
    BS = C * QB              # rows per bucket table
    assert BS <= 32704
    NG = _cdiv(NB, GROUPG)

    src = np.asarray(edge_index[0], dtype=np.int64)
    dst = np.asarray(edge_index[1], dtype=np.int64)
    perm = np.asarray(perm, dtype=np.int64)

    deg = (1.0 + np.bincount(dst, minlength=N)).astype(np.float32)
    dinv = (1.0 / np.sqrt(deg)).astype(np.float32)
    enorm = dinv[src] * dinv[dst]
    self_norm = dinv * dinv

    core = dst // SH
    ld = dst - core * SH
    blk = ld >> 7
    dloc = ld & 127
    sr = src // SH
    sl = src - sr * SH
    buck = sl // QB                       # bucket by local row
    sloc = (sr * QB + (sl - buck * QB)).astype(np.int64)  # row in bucket table

    gi = blk // GROUPG
    bg = blk - gi * GROUPG
    NSEG = NG * BUCK * GROUPG
    seg = ((core * NG + gi) * BUCK + buck) * GROUPG + bg
    order = np.argsort(seg, kind="stable")
    seg_s = seg[order]
    cnt = np.bincount(seg, minlength=C * NSEG).reshape(C, NG, BUCK, GROUPG)
    K4 = _cdiv(cnt, 128).max(axis=0)      # [NG, BUCK, GROUPG] chunks per seg
    kflat = K4.reshape(-1)
    chunk_base = np.concatenate([[0], np.cumsum(kflat)])
    TOT = int(kflat.sum())
    KG = K4.sum(axis=2)                   # [NG, BUCK] chunks per gather window

    seg_off = np.concatenate(
        [[0], np.cumsum(np.bincount(seg, minlength=C * NSEG))])
    rank = np.arange(E, dtype=np.int64) - seg_off[seg_s]
    lseg = seg_s % NSEG
    slot = chunk_base[lseg] * 128 + rank
    core_s = seg_s // NSEG

    idxbuf = np.full((C, TOT * 128), -1, np.int16)
    dstbuf = np.full((C, TOT * 128), 255.0, np.float32)  # 255 never matches iota
    idxbuf[core_s, slot] = sloc[order].astype(np.int16)
    dstbuf[core_s, slot] = dloc[order].astype(np.float32)

    # per-(group,bucket) window: interior segment pads fetch row 0 (enorm=0
    # kills their contribution); the tail after the last segment's real rows
    # stays -1 and is truncated via the count register. First WARM_WIN
    # windows fetch everything so gather slots' stale bytes are always
    # finite afterwards.
    gcnt = np.zeros((C, NG, BUCK), np.int32)
    warm_left = WARM_WIN if TRUNC_GATHER else (NG * BUCK + 1)
    for g_ in range(NG):
        for u_ in range(BUCK):
            kg = int(KG[g_, u_])
            if kg == 0:
                continue
            w0 = int(chunk_base[(g_ * BUCK + u_) * GROUPG]) * 128
            if warm_left > 0:
                win = idxbuf[:, w0:w0 + kg * 128]
                win[win < 0] = 0
                gcnt[:, g_, u_] = kg * 128
                warm_left -= 1
                continue
            nz = np.nonzero(K4[g_, u_])[0]
            lastb = int(nz[-1])
            off_last = int(K4[g_, u_, :lastb].sum()) * 128
            if off_last > 0:
                win = idxbuf[:, w0:w0 + off_last]
                win[win < 0] = 0
            gcnt[:, g_, u_] = off_last + cnt[:, g_, u_, lastb]
            zc = np.nonzero(gcnt[:, g_, u_] == 0)[0]
            if len(zc):
                idxbuf[zc, w0] = 0
                gcnt[zc, g_, u_] = 1

    idx_dev = np.tile(
        idxbuf.reshape(C, TOT, 8, 16).transpose(0, 3, 1, 2).reshape(C, 16, TOT * 8),
        (1, 8, 1),
    )  # [C, 128, TOT*8]
    dst_dev = dstbuf.reshape(C, TOT, 128).transpose(0, 2, 1).astype(
        ml_dtypes.bfloat16)
    gcnt_dev = np.ascontiguousarray(gcnt.reshape(C, NG * BUCK))

    dv_pad = np.zeros((C, NP), np.float32)
    dv_pad[:, :SH] = dinv.reshape(C, SH)
    dv_dev = dv_pad.reshape(C, NB, 128).transpose(0, 2, 1).copy()
    vd_pad = np.zeros((C, NP), np.float32)
    vd_pad[:, :SH] = 1.0
    vd_dev = vd_pad.reshape(C, NB, 128).transpose(0, 2, 1).copy()

    xT_pos = np.zeros((C, IN, NP), ml_dtypes.bfloat16)
    xT_neg = np.zeros((C, IN, NP), ml_dtypes.bfloat16)
    xr = x.reshape(C, SH, IN)
    xn = x[perm].reshape(C, SH, IN)
    for c in range(C):
        xT_pos[c, :, :SH] = xr[c].T
        xT_neg[c, :, :SH] = xn[c].T

    # per-block chunk counts, for test harness compat / loop structure
    K = np.zeros((NB, BUCK), np.int64)
    for b in range(NB):
        K[b] = K4[b // GROUPG, :, b % GROUPG]

    meta = dict(N=N, E=E, IN=IN, SH=SH, NB=NB, NP=NP, NPAD=NPAD, QB=QB,
                BS=BS, TOT=TOT, NG=NG, K=K, K4=K4, KG=KG,
                chunk_base=chunk_base)
    arrays = dict(idx_dev=idx_dev, dst_dev=dst_dev,
                  gcnt_dev=gcnt_dev, dv_dev=dv_dev, vd_dev=vd_dev,
                  xT_pos=xT_pos, xT_neg=xT_neg)
    return meta, arrays


# ----------------------------------------------------------------------------
# device program
# ----------------------------------------------------------------------------

def _build(meta, HID, OUT, bias1_nz, bias2_nz, bb_val, C):
    N, IN = meta["N"], meta["IN"]
    NB, NP, NG = meta["NB"], meta["NP"], meta["NG"]
    QB, BS, TOT = meta["QB"], meta["BS"], meta["TOT"]
    K4, KG, chunk_base = meta["K4"], meta["KG"], meta["chunk_base"]
    KI, KH = IN // 128, HID // 128
    assert OUT == 128, "discriminator path assumes OUT == 128"
    F1, F2 = 2 * HID, 2 * OUT
    KGMAX = int(KG.max())
    GDT = BF16 if GATHER_BF16 else F32   # layer-2 table dtype
    T1 = F8                              # layer-1 table dtype (e3m4)

    nc = bacc.Bacc("TRN2", target_bir_lowering=False, debug=False, num_devices=C,
                   num_swdge_queues=4)

    # inputs
    xtp = nc.dram_tensor("xtp", [IN, NP], BF16, kind="ExternalInput")
    xtn = nc.dram_tensor("xtn", [IN, NP], BF16, kind="ExternalInput")
    w1 = nc.dram_tensor("w1", [IN, HID], F32, kind="ExternalInput")
    w2 = nc.dram_tensor("w2", [HID, OUT], F32, kind="ExternalInput")
    wbt = nc.dram_tensor("wbt", [OUT, OUT], F32, kind="ExternalInput")
    idx_in = nc.dram_tensor("idx16", [128, TOT * 8], I16, kind="ExternalInput")
    dst_in = nc.dram_tensor("dstl", [128, TOT], BF16, kind="ExternalInput")
    gcnt_in = nc.dram_tensor("gcnt", [1, NG * BUCK], I32, kind="ExternalInput")
    dv_in = nc.dram_tensor("dinv", [128, NB], F32, kind="ExternalInput")
    vd_in = nc.dram_tensor("valid", [128, NB], F32, kind="ExternalInput")
    b1_in = nc.dram_tensor("b1bc", [128, F1], F32, kind="ExternalInput") if bias1_nz else None
    b2_in = nc.dram_tensor("b2bc", [128, F2], F32, kind="ExternalInput") if bias2_nz else None
    out = nc.dram_tensor("scores", [2, 128, NB], F32, kind="ExternalOutput")

    # internal DRAM
    hw1_sh = nc.dram_tensor("hw1_sh", [NP, F1], T1, kind="Internal")
    hw2_sh = nc.dram_tensor("hw2_sh", [NP, F2], GDT, kind="Internal")
    hw1_full = [nc.dram_tensor(f"hw1_full{j}", [BS, F1], T1, kind="Internal",
                               addr_space="Shared") for j in range(BUCK)]
    hw2_full = [nc.dram_tensor(f"hw2_full{j}", [BS, F2], GDT, kind="Internal",
                               addr_space="Shared") for j in range(BUCK)]
    h_sh = nc.dram_tensor("h_sh", [NP, F2], F32, kind="Internal")
    cs_in = nc.dram_tensor("cs_in", [128, 1], F32, kind="Internal")
    cs_out = nc.dram_tensor("cs_out", [128, 1], F32, kind="Internal",
                            addr_space="Shared")

    XSPAN = 4

    with tile.TileContext(nc) as tc:
        with tc.tile_pool(name="const", bufs=1) as cp, \
             tc.tile_pool(name="stream", bufs=3) as sp, \
             tc.tile_pool(name="meta", bufs=3) as mpp, \
             tc.tile_pool(name="mpool", bufs=MP_BUFS) as mp, \
             tc.tile_pool(name="gat", bufs=GAT_BUFS) as gp, \
             tc.tile_pool(name="psA", bufs=PSA_BUFS, space="PSUM") as psA, \
             tc.tile_pool(name="psT", bufs=2, space="PSUM") as psT, \
             tc.tile_pool(name="psH", bufs=2, space="PSUM") as psH, \
             tc.tile_pool(name="psC", bufs=1, space="PSUM") as psC:

            def allgather_bucket(shard, full, j, nm):
                if LOCAL_SIM:
                    for i in range(QB // 128):
                        tcp = sp.tile([128, full.shape[1]], GDT, tag="agcopy",
                                      name=f"agc_{nm}_{j}_{i}")
                        nc.sync.dma_start(
                            out=tcp[:],
                            in_=shard[j * QB + i * 128:j * QB + (i + 1) * 128, :])
                        nc.sync.dma_start(
                            out=full[i * 128:(i + 1) * 128, :], in_=tcp[:])
                    return
                cc = nc.gpsimd.collective_compute(
                    "AllGather", mybir.AluOpType.bypass,
                    replica_groups=[list(range(C))],
                    ins=[shard[j * QB:(j + 1) * QB, :].opt()],
                    outs=[full[:, :].opt()])
                for wi in warm_insts:
                    add_dep_helper(cc.ins, wi, True,
                                   "gather slots must be finite")

            # constants
            ident = cp.tile([128, 128], F32)
            make_identity(nc, ident[:])
            iota = cp.tile([128, 128], BF16)
            nc.gpsimd.iota(iota[:], pattern=[[1, 128]], base=0,
                           channel_multiplier=0,
                           allow_small_or_imprecise_dtypes=True)

            w1f = sp.tile([128, KI, HID], F32, tag="wstage", name="w1f")
            for k in range(KI):
                nc.sync.dma_start(out=w1f[:, k, :], in_=w1[k * 128:(k + 1) * 128, :])
            w1sb = cp.tile([128, KI, HID], BF16)
            nc.vector.tensor_copy(out=w1sb[:], in_=w1f[:])
            w2f = sp.tile([128, KH, OUT], F32, tag="wstage", name="w2f")
            for k in range(KH):
                nc.sync.dma_start(out=w2f[:, k, :], in_=w2[k * 128:(k + 1) * 128, :])
            w2sb = cp.tile([128, KH, OUT], BF16)
            nc.vector.tensor_copy(out=w2sb[:], in_=w2f[:])
            wbtsb = cp.tile([128, OUT], F32)
            nc.sync.dma_start(out=wbtsb[:], in_=wbt[:, :])
            dvsb = cp.tile([128, NB], F32)
            nc.sync.dma_start(out=dvsb[:], in_=dv_in[:, :])
            vdsb = cp.tile([128, NB], F32)
            nc.sync.dma_start(out=vdsb[:], in_=vd_in[:, :])
            gcsb = cp.tile([1, NG * BUCK], I32)
            nc.sync.dma_start(out=gcsb[:], in_=gcnt_in[:, :])
            b1sb = b2sb = None
            if bias1_nz:
                b1sb = cp.tile([128, F1], F32)
                nc.sync.dma_start(out=b1sb[:], in_=b1_in[:, :])
            if bias2_nz:
                b2sb = cp.tile([128, F2], F32)
                nc.sync.dma_start(out=b2sb[:], in_=b2_in[:, :])
            sc_pos = cp.tile([128, NB], F32, tag="scp")
            sc_neg = cp.tile([128, NB], F32, tag="scn")

            gregs = [nc.gpsimd.alloc_register(f"gcnt_r{i}") for i in range(4)]

            # warm the gather slots so skipped (-1) rows read finite stale data
            warm_insts = []
            for i in range(GAT_BUFS):
                gw = gp.tile([128, KGMAX, F1], T1, tag="gat", name=f"gwarm{i}")
                warm_insts.append(nc.vector.memset(gw[:], 0.0).ins)

            # ---------------- phase A: hw1 = x @ W1 (pos|neg) ----------------
            ag1_next = 0
            for sb0 in range(0, NB, XSPAN):
                span = min(XSPAN, NB - sb0)
                xp = sp.tile([128, KI, XSPAN * 128], BF16, tag="xtp")
                xn_t = sp.tile([128, KI, XSPAN * 128], BF16, tag="xtn")
                for k in range(KI):
                    nc.sync.dma_start(
                        out=xp[:, k, :span * 128],
                        in_=xtp[k * 128:(k + 1) * 128, sb0 * 128:(sb0 + span) * 128])
                    nc.sync.dma_start(
                        out=xn_t[:, k, :span * 128],
                        in_=xtn[k * 128:(k + 1) * 128, sb0 * 128:(sb0 + span) * 128])
                for j in range(span):
                    nb_ = sb0 + j
                    pa = psA.tile([128, F1], F32, tag="agg", space="PSUM")
                    for k in range(KI):
                        nc.tensor.matmul(
                            out=pa[:, 0:HID],
                            lhsT=xp[:, k, j * 128:(j + 1) * 128],
                            rhs=w1sb[:, k, :],
                            start=(k == 0), stop=(k == KI - 1))
                    for k in range(KI):
                        nc.tensor.matmul(
                            out=pa[:, HID:F1],
                            lhsT=xn_t[:, k, j * 128:(j + 1) * 128],
                            rhs=w1sb[:, k, :],
                            start=(k == 0), stop=(k == KI - 1))
                    hw1sb = sp.tile([128, F1], T1, tag="hw1sb")
                    nc.vector.tensor_scalar(
                        out=hw1sb[:], in0=pa[:],
                        scalar1=dvsb[:, nb_:nb_ + 1], scalar2=None,
                        op0=mybir.AluOpType.mult)
                    nc.sync.dma_start(out=hw1_sh[nb_ * 128:(nb_ + 1) * 128, :],
                                      in_=hw1sb[:])
                done_rows = (sb0 + span) * 128
                while ag1_next < BUCK and done_rows >= (ag1_next + 1) * QB:
                    allgather_bucket(hw1_sh, hw1_full[ag1_next], ag1_next, "h1")
                    ag1_next += 1
            while ag1_next < BUCK:
                allgather_bucket(hw1_sh, hw1_full[ag1_next], ag1_next, "h1")
                ag1_next += 1

            # ---------------- aggregation layers ----------------
            def agg_layer(layer):
                F = F1 if layer == 1 else F2
                DTY = T1 if layer == 1 else GDT
                fulls = hw1_full if layer == 1 else hw2_full
                shard = hw1_sh if layer == 1 else hw2_sh
                bsb = b1sb if layer == 1 else b2sb
                dl = ix = None
                cw = 0
                post_prev = None
                prev_b = -1
                ag2_next = [0]

                def maybe_kick_ag2(pb):
                    while (ag2_next[0] < BUCK
                           and (pb + 1 - 6) * 128 >= (ag2_next[0] + 1) * QB):
                        allgather_bucket(hw2_sh, hw2_full[ag2_next[0]],
                                         ag2_next[0], "h2")
                        ag2_next[0] += 1

                for gi_ in range(NG):
                    b0 = gi_ * GROUPG
                    be = min(NB, b0 + GROUPG)
                    if gi_ % MGG == 0:
                        ge = min(NG, gi_ + MGG)
                        cw = int(chunk_base[(gi_ * BUCK) * GROUPG])
                        gcols = int(chunk_base[(ge * BUCK) * GROUPG]) - cw
                        if gcols > 0:
                            dl = mpp.tile([128, gcols], BF16, tag="dl",
                                          name=f"dl{layer}_{gi_}")
                            ix = mpp.tile([128, gcols * 8], I16, tag="ix",
                                          name=f"ix{layer}_{gi_}")
                            nc.sync.dma_start(out=dl[:],
                                              in_=dst_in[:, cw:cw + gcols])
                            nc.sync.dma_start(
                                out=ix[:], in_=idx_in[:, cw * 8:(cw + gcols) * 8])
                    # grouped gathers for this window
                    gts = [None] * BUCK
                    for u in range(BUCK):
                        kg = int(KG[gi_, u])
                        if kg == 0:
                            continue
                        gt = gp.tile([128, KGMAX, F], DTY, tag="gat",
                                     name=f"gt{layer}_{gi_}_{u}")
                        cb_w = int(chunk_base[(gi_ * BUCK + u) * GROUPG]) - cw
                        if TRUNC_GATHER:
                            reg = gregs[u]
                            nc.gpsimd.reg_load(
                                reg, gcsb[0:1, gi_ * BUCK + u:gi_ * BUCK + u + 1])
                        else:
                            reg = kg * 128
                        nc.gpsimd.dma_gather(
                            out_ap=gt[:, :kg, :],
                            in_ap=fulls[u][:, :],
                            idxs_ap=ix[:, cb_w * 8:(cb_w + kg) * 8],
                            num_idxs=kg * 128,
                            num_idxs_reg=reg,
                            elem_size=F,
                            single_packet=(kg * 128 <= 1024),
                            queue_num=(gi_ * BUCK + u) % 4)
                        gts[u] = gt

                    # one wide one-hot selection build per (group, bucket)
                    mws = [None] * BUCK
                    for u in range(BUCK):
                        wg = int(KG[gi_, u])
                        if wg == 0:
                            continue
                        cb_w = int(chunk_base[(gi_ * BUCK + u) * GROUPG]) - cw
                        mwu = mp.tile([128, KGMAX, 128], DTY, tag="m",
                                      name=f"mw{layer}_{gi_}_{u}")
                        nc.vector.tensor_tensor(
                            out=mwu[:, :wg, :],
                            in0=iota[:].unsqueeze(1).to_broadcast([128, wg, 128]),
                            in1=dl[:, cb_w:cb_w + wg].unsqueeze(2)
                                .to_broadcast([128, wg, 128]),
                            op=mybir.AluOpType.is_equal)
                        mws[u] = mwu

                    for b in range(b0, be):
                        bg = b - b0
                        kb = int(K4[gi_, :, bg].sum())
                        selfr = sp.tile([128, F], DTY, tag="selfr",
                                        name=f"sf{layer}_{b}")
                        nc.sync.dma_start(
                            out=selfr[:], in_=shard[b * 128:(b + 1) * 128, :])
                        ps_agg = None
                        if kb > 0:
                            ps_agg = psA.tile([128, F1], F32, tag="agg",
                                              space="PSUM")
                            t = 0
                            for u in range(BUCK):
                                ku = int(K4[gi_, u, bg])
                                if ku == 0:
                                    continue
                                off = int(K4[gi_, u, :bg].sum())
                                for j in range(ku):
                                    nc.tensor.matmul(
                                        out=ps_agg[:, :F],
                                        lhsT=mws[u][:, off + j, :],
                                        rhs=gts[u][:, off + j, :],
                                        start=(t == 0), stop=(t == kb - 1))
                                    t += 1

                        def make_post(b=b, kb=kb, ps_agg=ps_agg, selfr=selfr):
                            def post():
                                hout = sp.tile([128, F], F32, tag="hout",
                                               name=f"ho{layer}_{b}")
                                if kb > 0:
                                    nc.vector.tensor_tensor(
                                        out=hout[:], in0=ps_agg[:, :F],
                                        in1=selfr[:], op=mybir.AluOpType.add)
                                    src_ap = hout
                                else:
                                    src_ap = selfr
                                # hout = (agg + selfr) * dinv  (+ relu, layer 1)
                                if layer == 1:
                                    nc.vector.tensor_scalar(
                                        out=hout[:], in0=src_ap[:],
                                        scalar1=dvsb[:, b:b + 1],
                                        scalar2=0.0,
                                        op0=mybir.AluOpType.mult,
                                        op1=mybir.AluOpType.max)
                                else:
                                    nc.vector.tensor_scalar(
                                        out=hout[:], in0=src_ap[:],
                                        scalar1=dvsb[:, b:b + 1],
                                        scalar2=None,
                                        op0=mybir.AluOpType.mult)
                                if bsb is not None:
                                    nc.vector.tensor_tensor(
                                        out=hout[:], in0=hout[:], in1=bsb[:],
                                        op=mybir.AluOpType.add)
                                if layer == 1:
                                    ps_tp = psT.tile([128, F1], F32, tag="tp",
                                                     space="PSUM",
                                                     name=f"tp{layer}_{b}")
                                    for k in range(2 * KH):
                                        nc.tensor.transpose(
                                            out=ps_tp[:, k * 128:(k + 1) * 128],
                                            in_=hout[:, k * 128:(k + 1) * 128],
                                            identity=ident[:])
                                    ts = sp.tile([128, F1], BF16, tag="ts",
                                                 name=f"ts{layer}_{b}")
                                    nc.scalar.copy(out=ts[:], in_=ps_tp[:])
                                    ps_h2 = psH.tile([128, F2], F32, tag="h2",
                                                     space="PSUM",
                                                     name=f"h2{layer}_{b}")
                                    for k in range(KH):
                                        nc.tensor.matmul(
                                            out=ps_h2[:, 0:OUT],
                                            lhsT=ts[:, k * 128:(k + 1) * 128],
                                            rhs=w2sb[:, k, :],
                                            start=(k == 0), stop=(k == KH - 1))
                                    for k in range(KH):
                                        nc.tensor.matmul(
                                            out=ps_h2[:, OUT:F2],
                                            lhsT=ts[:, (KH + k) * 128:(KH + k + 1) * 128],
                                            rhs=w2sb[:, k, :],
                                            start=(k == 0), stop=(k == KH - 1))
                                    hw2sb = sp.tile([128, F2], GDT, tag="hw2sb",
                                                    name=f"hw2sb{layer}_{b}")
                                    nc.scalar.activation(
                                        out=hw2sb[:], in_=ps_h2[:],
                                        func=mybir.ActivationFunctionType.Copy,
                                        scale=dvsb[:, b:b + 1])
                                    nc.sync.dma_start(
                                        out=hw2_sh[b * 128:(b + 1) * 128, :],
                                        in_=hw2sb[:])
                                else:
                                    if b == 0:
                                        ps_cs = psC.tile([128, 1], F32, tag="cs",
                                                         space="PSUM")
                                        agg_layer.cs = ps_cs
                                    else:
                                        ps_cs = agg_layer.cs
                                    nc.tensor.matmul(
                                        out=ps_cs[:], lhsT=hout[:, 0:OUT],
                                        rhs=vdsb[:, b:b + 1],
                                        start=(b == 0), stop=(b == NB - 1),
                                        skip_group_check=True)
                                    nc.sync.dma_start(
                                        out=h_sh[b * 128:(b + 1) * 128, :],
                                        in_=hout[:])
                            return post

                        if post_prev is not None:
                            post_prev()
                            if layer == 1:
                                maybe_kick_ag2(prev_b)
                        post_prev = make_post()
                        prev_b = b
                post_prev()
                if layer == 1:
                    maybe_kick_ag2(NB - 1)
                    while ag2_next[0] < BUCK:
                        allgather_bucket(hw2_sh, hw2_full[ag2_next[0]],
                                         ag2_next[0], "h2")
                        ag2_next[0] += 1

            agg_layer(1)
            agg_layer(2)

            # ---------------- summary s and v = Wb @ s ----------------
            cssb = sp.tile([128, 1], F32, tag="cssb")
            nc.vector.tensor_copy(out=cssb[:], in_=agg_layer.cs[:])
            nc.sync.dma_start(out=cs_in[:, :], in_=cssb[:])
            if LOCAL_SIM:
                nc.sync.dma_start(out=cs_out[:, :], in_=cssb[:])
            else:
                nc.gpsimd.collective_compute(
                    "AllReduce", mybir.AluOpType.add,
                    replica_groups=[list(range(C))],
                    ins=[cs_in[:, :].opt()], outs=[cs_out[:, :].opt()])
            csr = sp.tile([128, 1], F32, tag="csr")
            nc.sync.dma_start(out=csr[:], in_=cs_out[:, :])
            ssb = sp.tile([128, 1], F32, tag="ssb")
            nc.scalar.activation(out=ssb[:], in_=csr[:],
                                 func=mybir.ActivationFunctionType.Sigmoid,
                                 scale=1.0 / N)
            ps_v = psC.tile([128, 1], F32, tag="cs", space="PSUM")
            nc.tensor.matmul(out=ps_v[:], lhsT=wbtsb[:], rhs=ssb[:],
                             start=True, stop=True)
            vsq = sp.tile([128, 128], F32, tag="vsq")
            nc.vector.memset(vsq[:], 0.0)
            nc.vector.tensor_copy(out=vsq[:, 0:1], in_=ps_v[:])
            ps_vt = psT.tile([128, F1], F32, tag="tp", space="PSUM",
                             name="ps_vt")
            nc.tensor.transpose(out=ps_vt[:, 0:128], in_=vsq[:],
                                identity=ident[:])
            vt0 = sp.tile([128, 128], F32, tag="vt0")
            nc.vector.tensor_copy(out=vt0[:], in_=ps_vt[:, 0:128])
            ones = sp.tile([128, 128], F32, tag="ones")
            nc.vector.memset(ones[:], 1.0)
            ps_vr = psH.tile([128, F2], F32, tag="h2", space="PSUM",
                             name="ps_vr")
            nc.tensor.matmul(out=ps_vr[:, 0:128], lhsT=ones[:], rhs=vt0[:],
                             start=True, stop=True)
            nc.tensor.matmul(out=ps_vr[:, 128:F2], lhsT=ones[:], rhs=vt0[:],
                             start=True, stop=True)
            vrep = sp.tile([128, F2], F32, tag="vrep")
            nc.vector.tensor_copy(out=vrep[:], in_=ps_vr[:])

            # ---------------- scores ----------------
            for b in range(NB):
                hb = sp.tile([128, F2], F32, tag="hb", name=f"hb{b}")
                nc.sync.dma_start(out=hb[:],
                                  in_=h_sh[b * 128:(b + 1) * 128, :])
                scr = sp.tile([128, F2], F32, tag="scr", name=f"scr{b}")
                nc.vector.tensor_tensor(
                    out=scr[:], in0=hb[:], in1=vrep[:],
                    op=mybir.AluOpType.mult)
                nc.vector.reduce_sum(out=sc_pos[:, b:b + 1],
                                     in_=scr[:, 0:OUT],
                                     axis=mybir.AxisListType.X)
                nc.vector.reduce_sum(out=sc_neg[:, b:b + 1],
                                     in_=scr[:, OUT:F2],
                                     axis=mybir.AxisListType.X)
            nc.vector.tensor_scalar(
                out=sc_pos[:], in0=sc_pos[:], scalar1=float(bb_val),
                scalar2=None, op0=mybir.AluOpType.add)
            nc.vector.tensor_scalar(
                out=sc_neg[:], in0=sc_neg[:], scalar1=float(bb_val),
                scalar2=None, op0=mybir.AluOpType.add)
            nc.sync.dma_start(out=out[0, :, :], in_=sc_pos[:])
            nc.sync.dma_start(out=out[1, :, :], in_=sc_neg[:])

    nc.compile()
    return nc


# ----------------------------------------------------------------------------
# entry point
# ----------------------------------------------------------------------------

_CACHE = {}


def _get_program(meta, HID, OUT, bias1_nz, bias2_nz, bb_val, C):
    key = (meta["N"], meta["E"], meta["IN"], HID, OUT, bias1_nz, bias2_nz,
           float(bb_val), C, meta["TOT"], meta["K4"].tobytes())
    if key not in _CACHE:
        _CACHE[key] = _build(meta, HID, OUT, bias1_nz, bias2_nz, bb_val, C)
    return _CACHE[key]


def _make_in_maps(meta, arrs, W1, b1, W2, b2, Wb, C, bias1_nz, bias2_nz):
    in_maps = []
    for c in range(C):
        m = {
            "xtp": arrs["xT_pos"][c], "xtn": arrs["xT_neg"][c],
            "w1": W1, "w2": W2, "wbt": np.ascontiguousarray(Wb.T),
            "idx16": arrs["idx_dev"][c], "dstl": arrs["dst_dev"][c],
            "gcnt": arrs["gcnt_dev"][c][None, :],
            "dinv": arrs["dv_dev"][c], "valid": arrs["vd_dev"][c],
        }
        if bias1_nz:
            m["b1bc"] = np.tile(np.concatenate([b1, b1])[None, :], (128, 1))
        if bias2_nz:
            m["b2bc"] = np.tile(np.concatenate([b2, b2])[None, :], (128, 1))
        in_maps.append(m)
    return in_maps


def kernel(x, edge_index, perm, W1, b1, W2, b2, Wb, bb):
    C = 8
    x = np.asarray(x, np.float32)
    W1 = np.asarray(W1, np.float32)
    W2 = np.asarray(W2, np.float32)
    Wb = np.asarray(Wb, np.float32)
    b1 = np.asarray(b1, np.float32)
    b2 = np.asarray(b2, np.float32)
    bb_val = float(np.asarray(bb).reshape(-1)[0])
    N = x.shape[0]
    HID = W1.shape[1]
    OUT = W2.shape[1]

    meta, arrs = _prep(x, edge_index, perm, C)
    bias1_nz = bool(np.any(b1))
    bias2_nz = bool(np.any(b2))
    nc = _get_program(meta, HID, OUT, bias1_nz, bias2_nz, bb_val, C)
    in_maps = _make_in_maps(meta, arrs, W1, b1, W2, b2, Wb, C, bias1_nz, bias2_nz)

    res = bass_utils.run_bass_kernel_spmd(nc, in_maps, core_ids=list(range(C)))

    SH, NB = meta["SH"], meta["NB"]
    pos = np.empty((N, 1), np.float32)
    neg = np.empty((N, 1), np.float32)
    for c in range(C):
        sc = res.results[c]["scores"]
        pos[c * SH:(c + 1) * SH, 0] = sc[0].T.reshape(-1)[:SH]
        neg[c * SH:(c + 1) * SH, 0] = sc[1].T.reshape(-1)[:SH]
    return pos, neg
